# revision 12
# baseline (speedup 1.0000x reference)
"""AnchorGCN layer on 8 TRN2 NeuronCores.

reference:
    support = input @ W.T                         # [N, F]
    anchor_diff = adj / (colsum(adj) + eps)       # [N, A]
    node_diff   = adj / (rowsum(adj) + eps)       # [N, A]
    out = node_diff @ (anchor_diff.T @ support)   # [N, F]

Distributed formulation (rows of input/adj sharded across 8 cores):
    Q    = adj_shard.T @ [input_shard | 1]        # [A, F+1] per-core partial
           (col F of Q is the per-core colsum partial)
    Q^T  = AllReduce(Q^T)                         # only collective: ~530 KB
    msg  = (Q[:, :F] @ W.T) / (colsum + eps)      # [A, F], computed per core
    out  = (adj_shard @ msg) / (rowsum + eps)

The host pre-shards and pre-formats: inputs are shipped in bf16 (the
matmul compute dtype; f32 PSUM accumulation keeps the contraction
exact-ish), and adj is shipped in BOTH row layout (for Q, contracting
over nodes) and transposed layout (stationary operand of the final
matmul, contracting over anchors). The transposed copy is DMA'd during
the all-reduce window, when the DMA engines are otherwise idle.
Normalizations and the all-reduce run in f32.
"""

import numpy as np
import ml_dtypes

import concourse.bacc as bacc
import concourse.mybir as mybir
import concourse.tile as tile
from concourse.bass_utils import run_bass_kernel_spmd
from concourse.masks import make_identity
from concourse.bass import _add_dep_helper

F32 = mybir.dt.float32
BF16 = mybir.dt.bfloat16
COPY = mybir.ActivationFunctionType.Copy
AXF = mybir.AxisListType.X

N, A, F = 50000, 500, 256
EPS = 1e-12
CORES = 8
P = 128
APAD = 512            # anchors padded 500 -> 512 (4 chunks of 128)
FA = F + 1            # input gets a ones column appended
T_FULL = 49           # node tiles per core: 8*49*128 = 50176 >= 50000
ACH = APAD // P       # 4 anchor chunks
GRP = 8               # node tiles per DMA batch


def build(n_tiles: int = T_FULL, n_cores: int = CORES):
    nt = n_tiles
    rows = nt * P
    nc = bacc.Bacc("TRN2", target_bir_lowering=False, debug=False,
                   num_devices=n_cores)

    pk_d = nc.dram_tensor("packed", [rows, FA + APAD], BF16,
                          kind="ExternalInput")
    adjt_d = nc.dram_tensor("adjT", [APAD, rows], BF16, kind="ExternalInput")
    w_d = nc.dram_tensor("W", [F, F], F32, kind="ExternalInput")
    out_d = nc.dram_tensor("out", [rows, F], F32, kind="ExternalOutput")

    with tile.TileContext(nc) as tc:
        _build_tc(tc, nc, pk_d, adjt_d, w_d, out_d, nt, n_cores)
    nc.compile()
    return nc


def _build_tc(tc, nc, pk_d, adjt_d, w_d, out_d, nt, n_cores):
    ts = lambda i: slice(i * P, (i + 1) * P)
    PK = FA + APAD
    pk_t = pk_d.ap().rearrange("(t p) f -> p t f", p=P)     # [128, nt, 769]
    out_t = out_d.ap().rearrange("(t p) f -> p t f", p=P)   # [128, nt, 256]

    with tc.tile_pool(name="const", bufs=1) as const, \
         tc.tile_pool(name="persist", bufs=1) as persist, \
         tc.tile_pool(name="dram", bufs=1, space="DRAM") as dram:

        ident = const.tile([P, P], BF16)
        make_identity(nc, ident[:])
        ident32 = const.tile([P, P], F32)
        make_identity(nc, ident32[:])

        # ---- W -> W^T (bf16), laid out [fi%128, (fi_half, fo)] ----
        w_sb = const.tile([P, 2 * F], F32)
        nc.sync.dma_start(
            out=w_sb[:].rearrange("p (c f) -> p c f", c=2),
            in_=w_d.ap().rearrange("(c p) f -> p c f", p=P),
        )
        w_bf = const.tile([P, 2 * F], BF16)
        nc.scalar.copy(w_bf[:], w_sb[:])
        wt_bf = const.tile([P, 2 * F], BF16)
        w_bf3 = w_bf[:].rearrange("p (c f) -> p c f", c=2)
        wt_bf3 = wt_bf[:].rearrange("p (c f) -> p c f", c=2)
        with tc.tile_pool(name="wt_ps", bufs=2, space="PSUM") as wtp:
            for foh in range(2):
                for fih in range(2):
                    w_ps = wtp.tile([P, P], BF16, tag="w_ps", bufs=2)
                    nc.tensor.transpose(
                        w_ps[:], w_bf3[:, foh, ts(fih)], ident[:])
                    nc.vector.tensor_copy(wt_bf3[:, fih, ts(foh)], w_ps[:])

        adjT = persist.tile([P, ACH * nt * P], BF16)     # [a%128, (ac, t*n)]
        adjT4 = adjT[:].rearrange("p (c t n) -> p c t n", c=ACH, t=nt)
        rowsum = persist.tile([P, nt], F32)
        rrow = persist.tile([P, nt], F32)

        groups = [list(range(g, min(g + GRP, nt))) for g in range(0, nt, GRP)]

        # ================= loop 1: DMA + Q accumulation + rowsum =========
        with tc.tile_pool(name="qps", bufs=1, space="PSUM") as qps, \
             tc.tile_pool(name="qtps", bufs=1, space="PSUM") as qtps, \
             tc.tile_pool(name="l1", bufs=1) as l1:
            q_ps = [qps.tile([P, FA], F32, tag=f"q{i}", name=f"q_ps{i}")
                    for i in range(ACH)]
            for grp in groups:
                g0, gl = grp[0], len(grp)
                pk4 = l1.tile([P, gl * PK], BF16, tag="pk4", bufs=4)
                nc.sync.dma_start(
                    out=pk4[:].rearrange("p (t f) -> p t f", t=gl),
                    in_=pk_t[:, g0:g0 + gl, :])
                pk4v = pk4[:].rearrange("p (t f) -> p t f", t=gl)
                for j, t in enumerate(grp):
                    for ac in range(ACH):
                        nc.tensor.matmul(
                            q_ps[ac][:],
                            pk4v[:, j, FA + ac * P:FA + (ac + 1) * P],
                            pk4v[:, j, 0:FA],
                            start=(t == 0), stop=(t == nt - 1),
                        )
                    # rowsum, split across the two idle engines
                    if t % 2 == 0:
                        nc.vector.reduce_sum(
                            rowsum[:, t:t + 1], pk4v[:, j, FA:PK], axis=AXF)
                    else:
                        rtrash = l1.tile([P, APAD], BF16, tag="rt", bufs=3)
                        nc.scalar.activation(
                            rtrash[:], pk4v[:, j, FA:PK], COPY,
                            accum_out=rowsum[:, t:t + 1])

            nc.vector.tensor_scalar_add(rrow[:], rowsum[:], EPS)
            nc.vector.reciprocal(rrow[:], rrow[:])

            # ---- evacuate Q (cast bf16), transpose it, ship to AR ----
            q_sb = persist.tile([P, ACH * F], BF16)
            q_sb3 = q_sb[:].rearrange("p (c f) -> p c f", c=ACH)
            cs_sb = persist.tile([P, ACH], BF16)
            for ac in range(ACH):
                nc.vector.tensor_copy(q_sb3[:, ac, :], q_ps[ac][:, 0:F])
                nc.vector.tensor_copy(
                    cs_sb[:, ac:ac + 1], q_ps[ac][:, F:FA])

            # single contiguous AR payload: [p, (fh, a)] Q^T + 4 colsum cols
            ar_sb = persist.tile([P, 2 * APAD + ACH], BF16)
            qt3 = ar_sb[:, 0:2 * APAD].rearrange("p (c a) -> p c a", c=2)
            for fh in range(2):
                qt_ps = qtps.tile([P, APAD], BF16, tag="qt", bufs=2)
                for ac in range(ACH):
                    nc.tensor.transpose(
                        qt_ps[:, ts(ac)], q_sb3[:, ac, ts(fh)], ident[:])
                nc.vector.tensor_copy(qt3[:, fh, :], qt_ps[:])
            nc.vector.tensor_copy(ar_sb[:, 2 * APAD:], cs_sb[:])

            PAY = 2 * APAD + ACH
            q_in = dram.tile([P, PAY], BF16)
            q_out = dram.tile([n_cores * P, PAY], BF16, addr_space="Shared")
            qin_dma = nc.sync.dma_start(out=q_in[:, :], in_=ar_sb[:])
            # adj^T load rides the collective window: forced to queue
            # after the payload DMA so it cannot delay the trigger, and
            # it drains long before the gathered result is needed.
            adjt_dma = nc.sync.dma_start(
                out=adjT4,
                in_=adjt_d.ap().rearrange("(c p) n -> p c n", p=P))
            _add_dep_helper(adjt_dma.ins, qin_dma.ins, sync=True,
                            reason="adjT load must not precede AG payload")
            nc.gpsimd.collective_compute(
                "AllGather",
                mybir.AluOpType.bypass,
                replica_groups=[list(range(n_cores))],
                ins=[q_in.opt()],
                outs=[q_out.opt()],
            )

        # ---- gathered shards -> on-chip f32 sum (pipelined w/ DMA) ----
        PAY = 2 * APAD + ACH
        ag_sb = persist.tile([P, n_cores * PAY], BF16)
        ag3 = ag_sb[:].rearrange("p (r f) -> p r f", r=n_cores)
        qo3 = q_out[:, :].rearrange("(r p) f -> p r f", p=P)
        pair = []
        for k in range(4):
            nc.sync.dma_start(
                out=ag3[:, 2 * k:2 * k + 2, :], in_=qo3[:, 2 * k:2 * k + 2, :])
            pk = persist.tile([P, PAY], F32, name=f"pair{k}")
            eng = nc.vector if k % 2 == 0 else nc.gpsimd
            eng.tensor_tensor(
                pk[:], ag3[:, 2 * k, :], ag3[:, 2 * k + 1, :],
                op=mybir.AluOpType.add)
            pair.append(pk)
        acc_a = persist.tile([P, PAY], F32)
        acc_b = persist.tile([P, PAY], F32)
        nc.vector.tensor_tensor(
            acc_a[:], pair[0][:], pair[1][:], op=mybir.AluOpType.add)
        nc.gpsimd.tensor_tensor(
            acc_b[:], pair[2][:], pair[3][:], op=mybir.AluOpType.add)
        nc.vector.tensor_tensor(
            acc_a[:], acc_a[:], acc_b[:], op=mybir.AluOpType.add)
        rcol = persist.tile([P, ACH], F32)
        nc.vector.tensor_scalar_add(rcol[:], acc_a[:, 2 * APAD:], EPS)
        nc.vector.reciprocal(rcol[:], rcol[:])
        qt_bf = persist.tile([P, 2 * APAD], BF16)
        nc.scalar.copy(qt_bf[:], acc_a[:, 0:2 * APAD])
        qtb3 = qt_bf[:].rearrange("p (c a) -> p c a", c=2)

        msg_bf = persist.tile([P, ACH * F], BF16)   # [a%128, (ac, f)]
        msg3 = msg_bf[:].rearrange("p (c f) -> p c f", c=ACH)
        with tc.tile_pool(name="mps", bufs=1, space="PSUM") as mpsp:
            for ac in range(ACH):
                mps = mpsp.tile([P, F], F32, tag=f"m{ac}", name=f"mps{ac}")
                for fh in range(2):
                    nc.tensor.matmul(
                        mps[:], qtb3[:, fh, ts(ac)], wt_bf3[:, fh, :],
                        start=(fh == 0), stop=(fh == 1),
                    )
                nc.scalar.activation(
                    msg3[:, ac, 0:F], mps[:], COPY, scale=rcol[:, ac:ac + 1])

        # ====== loop 3: out = (adj @ msg) / (rowsum + eps) ======
        with tc.tile_pool(name="l3ps", bufs=6, space="PSUM") as l3ps, \
             tc.tile_pool(name="l3", bufs=1) as l3:
            for grp in groups:
                g0, gl = grp[0], len(grp)
                o4 = l3.tile([P, gl * F], F32, tag="o4", bufs=3)
                o4v = o4[:].rearrange("p (t f) -> p t f", t=gl)
                for j, t in enumerate(grp):
                    ops = l3ps.tile([P, F], F32, tag="ops", bufs=6)
                    for ac in range(ACH):
                        nc.tensor.matmul(
                            ops[:], adjT4[:, ac, t, :], msg3[:, ac, :],
                            start=(ac == 0), stop=(ac == ACH - 1),
                        )
                    if t % 2 == 0:
                        nc.scalar.activation(
                            o4v[:, j, :], ops[:], COPY, scale=rrow[:, t:t + 1])
                    else:
                        nc.vector.tensor_scalar_mul(
                            o4v[:, j, :], ops[:], rrow[:, t:t + 1])
                nc.sync.dma_start(
                    out=out_t[:, g0:g0 + gl, :], in_=o4v)


# ---------------------------------------------------------------------------
# host side
# ---------------------------------------------------------------------------

_NC_CACHE = {}


def _get_nc(n_tiles=T_FULL, n_cores=CORES):
    key = (n_tiles, n_cores)
    if key not in _NC_CACHE:
        _NC_CACHE[key] = build(n_tiles, n_cores)
    return _NC_CACHE[key]


def shard_inputs(input, adj, W, n_tiles=T_FULL, n_cores=CORES):
    bf16 = ml_dtypes.bfloat16
    n = input.shape[0]
    rows = n_tiles * P
    total = rows * n_cores
    pk = np.zeros((total, FA + APAD), dtype=bf16)
    pk[:n, :F] = input.astype(bf16)
    pk[:n, F] = 1.0
    pk[:n, FA:FA + A] = adj.astype(bf16)
    # padded rows get a 1 in a padded anchor column: rowsum=1 (no 0/0 in
    # the divide) while Q, colsum and real outputs are untouched.
    pk[n:, FA + A] = 1.0
    w = np.ascontiguousarray(W, dtype=np.float32)
    maps = []
    for c in range(n_cores):
        sl = pk[c * rows:(c + 1) * rows]
        maps.append({
            "packed": np.ascontiguousarray(sl),
            "adjT": np.ascontiguousarray(sl[:, FA:].T),
            "W": w,
        })
    return maps


def kernel(input, adj, W):
    input = np.asarray(input, dtype=np.float32)
    adj = np.asarray(adj, dtype=np.float32)
    W = np.asarray(W, dtype=np.float32)
    nc = _get_nc()
    in_maps = shard_inputs(input, adj, W)
    res = run_bass_kernel_spmd(nc, in_maps, core_ids=list(range(CORES)))
    out = np.concatenate([res.results[c]["out"] for c in range(CORES)], axis=0)
    return np.ascontiguousarray(out[:input.shape[0]])


# revision 13
# speedup vs baseline: 1.2212x; 1.2212x over previous
"""AnchorGCN layer on 8 TRN2 NeuronCores.

reference:
    support = input @ W.T                         # [N, F]
    anchor_diff = adj / (colsum(adj) + eps)       # [N, A]
    node_diff   = adj / (rowsum(adj) + eps)       # [N, A]
    out = node_diff @ (anchor_diff.T @ support)   # [N, F]

Distributed formulation (rows of input/adj sharded across 8 cores):
    Q    = adj_shard.T @ [input_shard | 1]        # [A, F+1] per-core partial
           (col F of Q is the per-core colsum partial)
    Q^T  = AllReduce(Q^T)                         # only collective: ~530 KB
    msg  = (Q[:, :F] @ W.T) / (colsum + eps)      # [A, F], computed per core
    out  = (adj_shard @ msg) / (rowsum + eps)

The host pre-shards and pre-formats: inputs are shipped in bf16 (the
matmul compute dtype; f32 PSUM accumulation keeps the contraction
exact-ish), and adj is shipped in BOTH row layout (for Q, contracting
over nodes) and transposed layout (stationary operand of the final
matmul, contracting over anchors). The transposed copy is DMA'd during
the all-reduce window, when the DMA engines are otherwise idle.
Normalizations and the all-reduce run in f32.
"""

import numpy as np
import ml_dtypes

import concourse.bacc as bacc
import concourse.mybir as mybir
import concourse.tile as tile
from concourse.bass_utils import run_bass_kernel_spmd
from concourse.masks import make_identity
from concourse.bass import _add_dep_helper

F32 = mybir.dt.float32
BF16 = mybir.dt.bfloat16
COPY = mybir.ActivationFunctionType.Copy
AXF = mybir.AxisListType.X

N, A, F = 50000, 500, 256
EPS = 1e-12
CORES = 8
P = 128
APAD = 512            # anchors padded 500 -> 512 (4 chunks of 128)
FA = F + 1            # input gets a ones column appended
T_FULL = 49           # node tiles per core: 8*49*128 = 50176 >= 50000
ACH = APAD // P       # 4 anchor chunks
GRP = 8               # node tiles per DMA batch


def build(n_tiles: int = T_FULL, n_cores: int = CORES):
    nt = n_tiles
    rows = nt * P
    nc = bacc.Bacc("TRN2", target_bir_lowering=False, debug=False,
                   num_devices=n_cores)

    pk_d = nc.dram_tensor("packed", [rows, FA + APAD], BF16,
                          kind="ExternalInput")
    adjt_d = nc.dram_tensor("adjT", [APAD, rows], BF16, kind="ExternalInput")
    w_d = nc.dram_tensor("W", [F, F], F32, kind="ExternalInput")
    out_d = nc.dram_tensor("out", [rows, F], F32, kind="ExternalOutput")

    with tile.TileContext(nc) as tc:
        _build_tc(tc, nc, pk_d, adjt_d, w_d, out_d, nt, n_cores)
    nc.compile()
    return nc


def _build_tc(tc, nc, pk_d, adjt_d, w_d, out_d, nt, n_cores):
    ts = lambda i: slice(i * P, (i + 1) * P)
    PK = FA + APAD
    pk_t = pk_d.ap().rearrange("(t p) f -> p t f", p=P)     # [128, nt, 769]
    out_t = out_d.ap().rearrange("(t p) f -> p t f", p=P)   # [128, nt, 256]

    with tc.tile_pool(name="const", bufs=1) as const, \
         tc.tile_pool(name="persist", bufs=1) as persist, \
         tc.tile_pool(name="dram", bufs=1, space="DRAM") as dram:

        ident = const.tile([P, P], BF16)
        make_identity(nc, ident[:])
        ident32 = const.tile([P, P], F32)
        make_identity(nc, ident32[:])

        # ---- W -> W^T (bf16), laid out [fi%128, (fi_half, fo)] ----
        w_sb = const.tile([P, 2 * F], F32)
        nc.sync.dma_start(
            out=w_sb[:].rearrange("p (c f) -> p c f", c=2),
            in_=w_d.ap().rearrange("(c p) f -> p c f", p=P),
        )
        w_bf = const.tile([P, 2 * F], BF16)
        nc.scalar.copy(w_bf[:], w_sb[:])
        wt_bf = const.tile([P, 2 * F], BF16)
        w_bf3 = w_bf[:].rearrange("p (c f) -> p c f", c=2)
        wt_bf3 = wt_bf[:].rearrange("p (c f) -> p c f", c=2)
        with tc.tile_pool(name="wt_ps", bufs=2, space="PSUM") as wtp:
            for foh in range(2):
                for fih in range(2):
                    w_ps = wtp.tile([P, P], BF16, tag="w_ps", bufs=2)
                    nc.tensor.transpose(
                        w_ps[:], w_bf3[:, foh, ts(fih)], ident[:])
                    nc.vector.tensor_copy(wt_bf3[:, fih, ts(foh)], w_ps[:])

        adjT = persist.tile([P, ACH * nt * P], BF16)     # [a%128, (ac, t*n)]
        adjT4 = adjT[:].rearrange("p (c t n) -> p c t n", c=ACH, t=nt)
        rowsum = persist.tile([P, nt], F32)
        rrow = persist.tile([P, nt], F32)

        groups = [list(range(g, min(g + GRP, nt))) for g in range(0, nt, GRP)]

        # ================= loop 1: DMA + Q accumulation + rowsum =========
        with tc.tile_pool(name="qps", bufs=1, space="PSUM") as qps, \
             tc.tile_pool(name="qtps", bufs=1, space="PSUM") as qtps, \
             tc.tile_pool(name="l1", bufs=1) as l1:
            q_ps = [qps.tile([P, FA], F32, tag=f"q{i}", name=f"q_ps{i}")
                    for i in range(ACH)]
            for grp in groups:
                g0, gl = grp[0], len(grp)
                pk4 = l1.tile([P, gl * PK], BF16, tag="pk4", bufs=4)
                nc.sync.dma_start(
                    out=pk4[:].rearrange("p (t f) -> p t f", t=gl),
                    in_=pk_t[:, g0:g0 + gl, :])
                pk4v = pk4[:].rearrange("p (t f) -> p t f", t=gl)
                for j, t in enumerate(grp):
                    for ac in range(ACH):
                        nc.tensor.matmul(
                            q_ps[ac][:],
                            pk4v[:, j, FA + ac * P:FA + (ac + 1) * P],
                            pk4v[:, j, 0:FA],
                            start=(t == 0), stop=(t == nt - 1),
                        )
                    # rowsum, split across the two idle engines
                    if t % 2 == 0:
                        nc.vector.reduce_sum(
                            rowsum[:, t:t + 1], pk4v[:, j, FA:PK], axis=AXF)
                    else:
                        rtrash = l1.tile([P, APAD], BF16, tag="rt", bufs=3)
                        nc.scalar.activation(
                            rtrash[:], pk4v[:, j, FA:PK], COPY,
                            accum_out=rowsum[:, t:t + 1])

            nc.vector.tensor_scalar_add(rrow[:], rowsum[:], EPS)
            nc.vector.reciprocal(rrow[:], rrow[:])

            # ---- evacuate Q (cast bf16), transpose it, ship to AR ----
            q_sb = persist.tile([P, ACH * F], BF16)
            q_sb3 = q_sb[:].rearrange("p (c f) -> p c f", c=ACH)
            cs_sb = persist.tile([P, ACH], BF16)
            for ac in range(ACH):
                nc.vector.tensor_copy(q_sb3[:, ac, :], q_ps[ac][:, 0:F])
                nc.vector.tensor_copy(
                    cs_sb[:, ac:ac + 1], q_ps[ac][:, F:FA])

            # single contiguous AR payload: [p, (fh, a)] Q^T + 4 colsum cols
            ar_sb = persist.tile([P, 2 * APAD + ACH], BF16)
            qt3 = ar_sb[:, 0:2 * APAD].rearrange("p (c a) -> p c a", c=2)
            for fh in range(2):
                qt_ps = qtps.tile([P, APAD], BF16, tag="qt", bufs=2)
                for ac in range(ACH):
                    nc.tensor.transpose(
                        qt_ps[:, ts(ac)], q_sb3[:, ac, ts(fh)], ident[:])
                nc.vector.tensor_copy(qt3[:, fh, :], qt_ps[:])
            nc.vector.tensor_copy(ar_sb[:, 2 * APAD:], cs_sb[:])

            PAY = 2 * APAD + ACH
            q_in = dram.tile([P, PAY], BF16)
            q_out = dram.tile([n_cores * P, PAY], BF16, addr_space="Shared")
            qin_dma = nc.sync.dma_start(out=q_in[:, :], in_=ar_sb[:])
            # adj^T load rides the collective window: forced to queue
            # after the payload DMA so it cannot delay the trigger, and
            # it drains long before the gathered result is needed.
            adjt_dma = nc.sync.dma_start(
                out=adjT4,
                in_=adjt_d.ap().rearrange("(c p) n -> p c n", p=P))
            _add_dep_helper(adjt_dma.ins, qin_dma.ins, sync=True,
                            reason="adjT load must not precede AG payload")
            nc.gpsimd.collective_compute(
                "AllGather",
                mybir.AluOpType.bypass,
                replica_groups=[list(range(n_cores))],
                ins=[q_in.opt()],
                outs=[q_out.opt()],
            )

        # ---- gathered shards summed inside the msg matmul's PSUM ----
        PAY = 2 * APAD + ACH
        ag_sb = persist.tile([P, n_cores * PAY], BF16)
        ag3 = ag_sb[:].rearrange("p (r f) -> p r f", r=n_cores)
        qo3 = q_out[:, :].rearrange("(r p) f -> p r f", p=P)
        for k in range(4):
            nc.sync.dma_start(
                out=ag3[:, 2 * k:2 * k + 2, :], in_=qo3[:, 2 * k:2 * k + 2, :])
        # total colsum: one strided reduce over the rank axis
        csview = ag3[:, :, 2 * APAD:].rearrange("p r f -> p f r")
        rcol = persist.tile([P, ACH], F32)
        nc.vector.reduce_sum(
            rcol[:].rearrange("p (f o) -> p f o", o=1), csview, axis=AXF)
        nc.vector.tensor_scalar_add(rcol[:], rcol[:], EPS)
        nc.vector.reciprocal(rcol[:], rcol[:])

        msg_bf = persist.tile([P, ACH * F], BF16)   # [a%128, (ac, f)]
        msg3 = msg_bf[:].rearrange("p (c f) -> p c f", c=ACH)
        with tc.tile_pool(name="mps", bufs=1, space="PSUM") as mpsp:
            mps = [mpsp.tile([P, F], F32, tag=f"m{i}", name=f"mps{i}")
                   for i in range(ACH)]
            for r in range(n_cores):
                shard = ag3[:, r, 0:2 * APAD].rearrange(
                    "p (c a) -> p c a", c=2)
                for ac in range(ACH):
                    for fh in range(2):
                        nc.tensor.matmul(
                            mps[ac][:], shard[:, fh, ts(ac)], wt_bf3[:, fh, :],
                            start=(r == 0 and fh == 0),
                            stop=(r == n_cores - 1 and fh == 1),
                        )
            for ac in range(ACH):
                nc.scalar.activation(
                    msg3[:, ac, 0:F], mps[ac][:], COPY,
                    scale=rcol[:, ac:ac + 1])

        # ====== loop 3: out = (adj @ msg) / (rowsum + eps) ======
        with tc.tile_pool(name="l3ps", bufs=6, space="PSUM") as l3ps, \
             tc.tile_pool(name="l3", bufs=1) as l3:
            for grp in groups:
                g0, gl = grp[0], len(grp)
                o4 = l3.tile([P, gl * F], F32, tag="o4", bufs=3)
                o4v = o4[:].rearrange("p (t f) -> p t f", t=gl)
                for j, t in enumerate(grp):
                    ops = l3ps.tile([P, F], F32, tag="ops", bufs=6)
                    for ac in range(ACH):
                        nc.tensor.matmul(
                            ops[:], adjT4[:, ac, t, :], msg3[:, ac, :],
                            start=(ac == 0), stop=(ac == ACH - 1),
                        )
                    if t % 2 == 0:
                        nc.scalar.activation(
                            o4v[:, j, :], ops[:], COPY, scale=rrow[:, t:t + 1])
                    else:
                        nc.vector.tensor_scalar_mul(
                            o4v[:, j, :], ops[:], rrow[:, t:t + 1])
                nc.sync.dma_start(
                    out=out_t[:, g0:g0 + gl, :], in_=o4v)


# ---------------------------------------------------------------------------
# host side
# ---------------------------------------------------------------------------

_NC_CACHE = {}


def _get_nc(n_tiles=T_FULL, n_cores=CORES):
    key = (n_tiles, n_cores)
    if key not in _NC_CACHE:
        _NC_CACHE[key] = build(n_tiles, n_cores)
    return _NC_CACHE[key]


def shard_inputs(input, adj, W, n_tiles=T_FULL, n_cores=CORES):
    bf16 = ml_dtypes.bfloat16
    n = input.shape[0]
    rows = n_tiles * P
    total = rows * n_cores
    pk = np.zeros((total, FA + APAD), dtype=bf16)
    pk[:n, :F] = input.astype(bf16)
    pk[:n, F] = 1.0
    pk[:n, FA:FA + A] = adj.astype(bf16)
    # padded rows get a 1 in a padded anchor column: rowsum=1 (no 0/0 in
    # the divide) while Q, colsum and real outputs are untouched.
    pk[n:, FA + A] = 1.0
    w = np.ascontiguousarray(W, dtype=np.float32)
    maps = []
    for c in range(n_cores):
        sl = pk[c * rows:(c + 1) * rows]
        maps.append({
            "packed": np.ascontiguousarray(sl),
            "adjT": np.ascontiguousarray(sl[:, FA:].T),
            "W": w,
        })
    return maps


def kernel(input, adj, W):
    input = np.asarray(input, dtype=np.float32)
    adj = np.asarray(adj, dtype=np.float32)
    W = np.asarray(W, dtype=np.float32)
    nc = _get_nc()
    in_maps = shard_inputs(input, adj, W)
    res = run_bass_kernel_spmd(nc, in_maps, core_ids=list(range(CORES)))
    out = np.concatenate([res.results[c]["out"] for c in range(CORES)], axis=0)
    return np.ascontiguousarray(out[:input.shape[0]])


# revision 18
# speedup vs baseline: 1.2314x; 1.0084x over previous
"""AnchorGCN layer on 8 TRN2 NeuronCores.

reference:
    support = input @ W.T                         # [N, F]
    anchor_diff = adj / (colsum(adj) + eps)       # [N, A]
    node_diff   = adj / (rowsum(adj) + eps)       # [N, A]
    out = node_diff @ (anchor_diff.T @ support)   # [N, F]

Distributed formulation (rows of input/adj sharded across 8 cores):
    Q    = adj_shard.T @ [input_shard | 1]        # [A, F+1] per-core partial
           (col F of Q is the per-core colsum partial)
    Q^T  = AllReduce(Q^T)                         # only collective: ~530 KB
    msg  = (Q[:, :F] @ W.T) / (colsum + eps)      # [A, F], computed per core
    out  = (adj_shard @ msg) / (rowsum + eps)

The host pre-shards and pre-formats: inputs are shipped in bf16 (the
matmul compute dtype; f32 PSUM accumulation keeps the contraction
exact-ish), and adj is shipped in BOTH row layout (for Q, contracting
over nodes) and transposed layout (stationary operand of the final
matmul, contracting over anchors). The transposed copy is DMA'd during
the all-reduce window, when the DMA engines are otherwise idle.
Normalizations and the all-reduce run in f32.
"""

import numpy as np
import ml_dtypes

import concourse.bacc as bacc
import concourse.mybir as mybir
import concourse.tile as tile
from concourse.bass_utils import run_bass_kernel_spmd
from concourse.masks import make_identity
from concourse.bass import _add_dep_helper

F32 = mybir.dt.float32
BF16 = mybir.dt.bfloat16
COPY = mybir.ActivationFunctionType.Copy
AXF = mybir.AxisListType.X

N, A, F = 50000, 500, 256
EPS = 1e-12
CORES = 8
P = 128
APAD = 512            # anchors padded 500 -> 512 (4 chunks of 128)
FA = F + 1            # input gets a ones column appended
T_FULL = 49           # node tiles per core: 8*49*128 = 50176 >= 50000
ACH = APAD // P       # 4 anchor chunks
GRP = 8               # node tiles per DMA batch


def build(n_tiles: int = T_FULL, n_cores: int = CORES):
    nt = n_tiles
    rows = nt * P
    nc = bacc.Bacc("TRN2", target_bir_lowering=False, debug=False,
                   num_devices=n_cores)

    pk_d = nc.dram_tensor("packed", [rows, FA + APAD], BF16,
                          kind="ExternalInput")
    adjt_d = nc.dram_tensor("adjT", [APAD, rows], BF16, kind="ExternalInput")
    w_d = nc.dram_tensor("W", [F, F], F32, kind="ExternalInput")
    out_d = nc.dram_tensor("out", [rows, F], F32, kind="ExternalOutput")

    with tile.TileContext(nc) as tc:
        _build_tc(tc, nc, pk_d, adjt_d, w_d, out_d, nt, n_cores)
    nc.compile()
    return nc


def _build_tc(tc, nc, pk_d, adjt_d, w_d, out_d, nt, n_cores):
    ts = lambda i: slice(i * P, (i + 1) * P)
    PK = FA + APAD
    pk_t = pk_d.ap().rearrange("(t p) f -> p t f", p=P)     # [128, nt, 769]
    out_t = out_d.ap().rearrange("(t p) f -> p t f", p=P)   # [128, nt, 256]

    with tc.tile_pool(name="const", bufs=1) as const, \
         tc.tile_pool(name="persist", bufs=1) as persist, \
         tc.tile_pool(name="dram", bufs=1, space="DRAM") as dram:

        ident = const.tile([P, P], BF16)
        make_identity(nc, ident[:])
        ident32 = const.tile([P, P], F32)
        make_identity(nc, ident32[:])

        # ---- W -> W^T (bf16), laid out [fi%128, (fi_half, fo)] ----
        w_sb = const.tile([P, 2 * F], F32)
        nc.gpsimd.dma_start(
            out=w_sb[:].rearrange("p (c f) -> p c f", c=2),
            in_=w_d.ap().rearrange("(c p) f -> p c f", p=P),
        )
        w_bf = const.tile([P, 2 * F], BF16)
        nc.scalar.copy(w_bf[:], w_sb[:])
        wt_bf = const.tile([P, 2 * F], BF16)
        w_bf3 = w_bf[:].rearrange("p (c f) -> p c f", c=2)
        wt_bf3 = wt_bf[:].rearrange("p (c f) -> p c f", c=2)
        with tc.tile_pool(name="wt_ps", bufs=2, space="PSUM") as wtp:
            for foh in range(2):
                for fih in range(2):
                    w_ps = wtp.tile([P, P], BF16, tag="w_ps", bufs=2)
                    nc.tensor.transpose(
                        w_ps[:], w_bf3[:, foh, ts(fih)], ident[:])
                    nc.vector.tensor_copy(wt_bf3[:, fih, ts(foh)], w_ps[:])

        adjT = persist.tile([P, ACH * nt * P], BF16)     # [a%128, (ac, t*n)]
        adjT4 = adjT[:].rearrange("p (c t n) -> p c t n", c=ACH, t=nt)
        rowsum = persist.tile([P, nt], F32)
        rrow = persist.tile([P, nt], F32)

        def make_groups(sizes):
            out, pos = [], 0
            for s in sizes:
                out.append(list(range(pos, pos + s)))
                pos += s
            assert pos == nt
            return out

        if nt == T_FULL:
            groups = make_groups([2, 2, 4] + [8] * 5 + [1])
            ogroups = make_groups([8, 12, 14, 15])
        else:
            groups = make_groups([nt])
            ogroups = groups

        # ================= loop 1: DMA + Q accumulation + rowsum =========
        with tc.tile_pool(name="qps", bufs=1, space="PSUM") as qps, \
             tc.tile_pool(name="qtps", bufs=1, space="PSUM") as qtps, \
             tc.tile_pool(name="l1", bufs=1) as l1:
            q_ps = [qps.tile([P, FA], F32, tag=f"q{i}", name=f"q_ps{i}")
                    for i in range(ACH)]
            for grp in groups:
                g0, gl = grp[0], len(grp)
                pk4 = l1.tile([P, gl * PK], BF16, tag="pk4", bufs=4)
                nc.sync.dma_start(
                    out=pk4[:].rearrange("p (t f) -> p t f", t=gl),
                    in_=pk_t[:, g0:g0 + gl, :])
                pk4v = pk4[:].rearrange("p (t f) -> p t f", t=gl)
                for j, t in enumerate(grp):
                    for ac in range(ACH):
                        nc.tensor.matmul(
                            q_ps[ac][:],
                            pk4v[:, j, FA + ac * P:FA + (ac + 1) * P],
                            pk4v[:, j, 0:FA],
                            start=(t == 0), stop=(t == nt - 1),
                        )
                    # rowsum, split across the two idle engines
                    if t % 2 == 0:
                        nc.vector.reduce_sum(
                            rowsum[:, t:t + 1], pk4v[:, j, FA:PK], axis=AXF)
                    else:
                        rtrash = l1.tile([P, APAD], BF16, tag="rt", bufs=3)
                        nc.scalar.activation(
                            rtrash[:], pk4v[:, j, FA:PK], COPY,
                            accum_out=rowsum[:, t:t + 1])

            nc.vector.tensor_scalar_add(rrow[:], rowsum[:], EPS)
            nc.vector.reciprocal(rrow[:], rrow[:])

            # ---- evacuate Q (cast bf16), transpose it, ship to AR ----
            q_sb = persist.tile([P, ACH * F], BF16)
            q_sb3 = q_sb[:].rearrange("p (c f) -> p c f", c=ACH)
            cs_sb = persist.tile([P, ACH], BF16)
            for ac in range(ACH):
                nc.vector.tensor_copy(q_sb3[:, ac, :], q_ps[ac][:, 0:F])
                nc.vector.tensor_copy(
                    cs_sb[:, ac:ac + 1], q_ps[ac][:, F:FA])

            # single contiguous AR payload: [p, (fh, a)] Q^T + 4 colsum cols
            ar_sb = persist.tile([P, 2 * APAD + ACH], BF16)
            qt3 = ar_sb[:, 0:2 * APAD].rearrange("p (c a) -> p c a", c=2)
            for fh in range(2):
                qt_ps = qtps.tile([P, APAD], BF16, tag="qt", bufs=2)
                for ac in range(ACH):
                    nc.tensor.transpose(
                        qt_ps[:, ts(ac)], q_sb3[:, ac, ts(fh)], ident[:])
                nc.vector.tensor_copy(qt3[:, fh, :], qt_ps[:])
            nc.vector.tensor_copy(ar_sb[:, 2 * APAD:], cs_sb[:])

            PAY = 2 * APAD + ACH
            q_in = dram.tile([P, PAY], BF16)
            q_out = dram.tile([n_cores * P, PAY], BF16, addr_space="Shared")
            qin_dma = nc.sync.dma_start(out=q_in[:, :], in_=ar_sb[:])
            # adj^T load rides the collective window: forced to queue
            # after the payload DMA so it cannot delay the trigger, and
            # it drains long before the gathered result is needed.
            adjt_dma = nc.sync.dma_start(
                out=adjT4,
                in_=adjt_d.ap().rearrange("(c p) n -> p c n", p=P))
            _add_dep_helper(adjt_dma.ins, qin_dma.ins, sync=True,
                            reason="adjT load must not precede AG payload")
            nc.gpsimd.collective_compute(
                "AllGather",
                mybir.AluOpType.bypass,
                replica_groups=[list(range(n_cores))],
                ins=[q_in.opt()],
                outs=[q_out.opt()],
            )

        # ---- gathered shards summed inside the msg matmul's PSUM ----
        PAY = 2 * APAD + ACH
        ag_sb = persist.tile([P, n_cores * PAY], BF16)
        ag3 = ag_sb[:].rearrange("p (r f) -> p r f", r=n_cores)
        qo3 = q_out[:, :].rearrange("(r p) f -> p r f", p=P)
        for k in range(2):
            nc.sync.dma_start(
                out=ag3[:, 4 * k:4 * k + 4, :], in_=qo3[:, 4 * k:4 * k + 4, :])
        # total colsum: one strided reduce over the rank axis
        csview = ag3[:, :, 2 * APAD:].rearrange("p r f -> p f r")
        rcol = persist.tile([P, ACH], F32)
        nc.vector.reduce_sum(
            rcol[:].rearrange("p (f o) -> p f o", o=1), csview, axis=AXF)
        nc.vector.tensor_scalar_add(rcol[:], rcol[:], EPS)
        nc.vector.reciprocal(rcol[:], rcol[:])

        msg_bf = persist.tile([P, ACH * F], BF16)   # [a%128, (ac, f)]
        msg3 = msg_bf[:].rearrange("p (c f) -> p c f", c=ACH)
        with tc.tile_pool(name="mps", bufs=1, space="PSUM") as mpsp:
            mps = [mpsp.tile([P, F], F32, tag=f"m{i}", name=f"mps{i}")
                   for i in range(ACH)]
            for r in range(n_cores):
                shard = ag3[:, r, 0:2 * APAD].rearrange(
                    "p (c a) -> p c a", c=2)
                for ac in range(ACH):
                    for fh in range(2):
                        nc.tensor.matmul(
                            mps[ac][:], shard[:, fh, ts(ac)], wt_bf3[:, fh, :],
                            start=(r == 0 and fh == 0),
                            stop=(r == n_cores - 1 and fh == 1),
                        )
            for ac in range(ACH):
                nc.scalar.activation(
                    msg3[:, ac, 0:F], mps[ac][:], COPY,
                    scale=rcol[:, ac:ac + 1])

        # ====== loop 3: out = (adj @ msg) / (rowsum + eps) ======
        with tc.tile_pool(name="l3ps", bufs=8, space="PSUM") as l3ps, \
             tc.tile_pool(name="l3", bufs=1) as l3:
            for grp in ogroups:
                g0, gl = grp[0], len(grp)
                o4 = l3.tile([P, gl * F], F32, tag="o4", bufs=2)
                o4v = o4[:].rearrange("p (t f) -> p t f", t=gl)
                for j, t in enumerate(grp):
                    ops = l3ps.tile([P, F], F32, tag="ops", bufs=8)
                    for ac in range(ACH):
                        nc.tensor.matmul(
                            ops[:], adjT4[:, ac, t, :], msg3[:, ac, :],
                            start=(ac == 0), stop=(ac == ACH - 1),
                        )
                    if t % 2 == 0:
                        nc.scalar.activation(
                            o4v[:, j, :], ops[:], COPY, scale=rrow[:, t:t + 1])
                    else:
                        nc.vector.tensor_scalar_mul(
                            o4v[:, j, :], ops[:], rrow[:, t:t + 1])
                nc.sync.dma_start(
                    out=out_t[:, g0:g0 + gl, :], in_=o4v)


# ---------------------------------------------------------------------------
# host side
# ---------------------------------------------------------------------------

_NC_CACHE = {}


def _get_nc(n_tiles=T_FULL, n_cores=CORES):
    key = (n_tiles, n_cores)
    if key not in _NC_CACHE:
        _NC_CACHE[key] = build(n_tiles, n_cores)
    return _NC_CACHE[key]


def shard_inputs(input, adj, W, n_tiles=T_FULL, n_cores=CORES):
    bf16 = ml_dtypes.bfloat16
    n = input.shape[0]
    rows = n_tiles * P
    total = rows * n_cores
    pk = np.zeros((total, FA + APAD), dtype=bf16)
    pk[:n, :F] = input.astype(bf16)
    pk[:n, F] = 1.0
    pk[:n, FA:FA + A] = adj.astype(bf16)
    # padded rows get a 1 in a padded anchor column: rowsum=1 (no 0/0 in
    # the divide) while Q, colsum and real outputs are untouched.
    pk[n:, FA + A] = 1.0
    w = np.ascontiguousarray(W, dtype=np.float32)
    maps = []
    for c in range(n_cores):
        sl = pk[c * rows:(c + 1) * rows]
        maps.append({
            "packed": np.ascontiguousarray(sl),
            "adjT": np.ascontiguousarray(sl[:, FA:].T),
            "W": w,
        })
    return maps


def kernel(input, adj, W):
    input = np.asarray(input, dtype=np.float32)
    adj = np.asarray(adj, dtype=np.float32)
    W = np.asarray(W, dtype=np.float32)
    nc = _get_nc()
    in_maps = shard_inputs(input, adj, W)
    res = run_bass_kernel_spmd(nc, in_maps, core_ids=list(range(CORES)))
    out = np.concatenate([res.results[c]["out"] for c in range(CORES)], axis=0)
    return np.ascontiguousarray(out[:input.shape[0]])
